# revision 22
# baseline (speedup 1.0000x reference)
"""Scatter-add (col2im at random query corners) on 8 Trainium2 NeuronCores.

Problem: out[t,c,h+dh,w+dw] += patches[n,0,c,dh,dw] for each query n at
corner (t,h,w), on top of the vid2fill base. PT=1, so every patch touches
exactly one frame: shard by frame pairs (core k owns frames 2k, 2k+1); the
cores are fully independent, no collective needed.

Strategy ("depth-class compaction"): the host computes each output
element's contributor count (its depth d), groups output elements by d,
and lays the patch values out per class d as layer-major [128, d, n_d/128]
blocks — a pure permutation/padding of the input values (quantized to
fp16; the 2e-2 tolerance leaves ~16x margin over fp16's ~1.2e-3 worst
case). The device, per layer, streams one contiguous DMA load and performs
one in-place full-partition vector add over the layer slice, then stores
each class's slice once its last layer has folded. Every addition of the
scatter-add happens on-device as a dense, full-bandwidth op — the
memory-regime optimum (device traffic ~= fp16 patch bytes + output bytes).

fp16 halves DMA traffic vs f32; trailing small layers are merged into one
load block and deep small classes into one store so every DMA keeps >=512
contiguous bytes per descriptor (below that the DMA bus pays a 2x latency
multiplier).

Elements with depth 0 (base only) and depth 1 (a single contribution, no
addition required anywhere) are routed by the host during unpermutation at
full f32 precision.
"""

import sys
from contextlib import ExitStack

for _p in ("/opt/trn_rl_repo", "/root/.axon_site/_ro/trn_rl_repo"):
    if _p not in sys.path:
        sys.path.append(_p)

import numpy as np

import concourse.bass as bass
from concourse import mybir
from concourse.bass_utils import run_bass_kernel_spmd

T, C, H, W = 16, 3, 512, 512
PS, PT = 7, 1
NCORES = 8
FPC = T // NCORES          # frames per core
NPIX = FPC * H * W         # pixels per core
NELEM = NPIX * C           # channels-last elements per core
P = 128                    # SBUF partitions
MIN_DEV_CLASS = 2          # depth-1 elements need no addition; host routes them
MERGE_W = 512              # merge layer loads narrower than this (cols)
MERGE_STORE_COLS = 256     # merge deepest classes until stores are this wide


def _prep_core(patches_k, q_k, base_k):
    """Per-core contribution stream + depth classes (host, pure indexing)."""
    h = q_k[:, 1]
    w = q_k[:, 2]
    lt = q_k[:, 0]

    dh = np.arange(PS, dtype=np.int64)
    dw = np.arange(PS, dtype=np.int64)
    ch = np.arange(C, dtype=np.int64)
    # channels-last element index, axis order (n, c, dh, dw) = patches order
    pix = (lt[:, None, None] * H + (h[:, None, None] + dh[None, :, None])) * W + (
        w[:, None, None] + dw[None, None, :]
    )
    e = (pix[:, None, :, :] * C + ch[None, :, None, None]).reshape(-1)
    v = patches_k.reshape(-1)

    if base_k is not None:
        # fold the base video in as one extra contribution per element
        e = np.concatenate([e, np.arange(NELEM, dtype=np.int64)])
        v = np.concatenate([v, base_k.reshape(-1)])

    cnt = np.bincount(e, minlength=NELEM)          # depth per element
    order = np.argsort(e, kind="stable")
    es = e[order]
    vs = v[order]
    grp_start = np.cumsum(cnt) - cnt
    rank = np.arange(es.shape[0], dtype=np.int64) - grp_start[es]

    elem_class = cnt
    max_d = int(cnt.max()) if cnt.size else 0
    class_sizes = np.bincount(elem_class, minlength=max_d + 1)
    pos_in_class = np.empty(NELEM, dtype=np.int64)
    cls_order = np.argsort(elem_class, kind="stable")
    cls_starts = np.cumsum(class_sizes) - class_sizes
    pos_in_class[cls_order] = np.arange(NELEM, dtype=np.int64) - cls_starts[
        elem_class[cls_order]
    ]
    return es, vs, rank, elem_class, pos_in_class, class_sizes


def _layout(class_list):
    """Layer-major fp16 layout. class_list must be sorted descending by depth.

    Returns a dict with:
      cl          class list (descending depth)
      A[d]        acc-region column of class d
      W0          total acc columns
      W[l]        columns of layer l (classes with depth >= l+1), l=1..maxd-1
      sb_off[l]   SBUF column of layer l's staging slice (l>=1); acc at 0
      totf        total SBUF columns (acc + staging)
      BO[l]       DRAM element offset of layer l's partition-0 row (l=0..)
      RW[l]       DRAM row stride (elements) of the block holding layer l
      loads       [(dram_base, row_w, sb_col, layers)] one DMA per entry
      stores      [(gate_layer, sb_col, ncols, out_base)] one DMA per entry
      vals_len    DRAM input elements (fp16)
      out_len     DRAM output elements (fp16)
      out_off[d]  DRAM output element offset of class d's segment
    """
    cl = list(class_list)
    A = {}
    off = 0
    for d, c in cl:
        A[d] = off
        off += c
    W0 = off
    maxd = cl[0][0]
    Wl = {l: sum(c for d, c in cl if d >= l + 1) for l in range(1, maxd)}

    sb_off = {}
    off = W0
    for l in range(1, maxd):
        sb_off[l] = off
        off += Wl[l]
    totf = off

    # --- load blocks ---
    # block0 holds layers 0+1 (every device element's first two values),
    # row-interleaved so one DMA fills acc + the first staging slice.
    BO = {0: 0, 1: W0}
    RW = {0: W0 + Wl[1], 1: W0 + Wl[1]}
    loads = [(0, W0 + Wl[1], 0, (0, 1))]
    base = 128 * (W0 + Wl[1])
    rest = [l for l in range(2, maxd)]
    head = [l for l in rest if Wl[l] >= MERGE_W]
    tail = [l for l in rest if Wl[l] < MERGE_W]
    for l in head:
        BO[l] = base
        RW[l] = Wl[l]
        loads.append((base, Wl[l], sb_off[l], (l,)))
        base += 128 * Wl[l]
    if tail:
        tw = sum(Wl[l] for l in tail)
        pref = 0
        for l in tail:
            BO[l] = base + pref
            RW[l] = tw
            pref += Wl[l]
        loads.append((base, tw, sb_off[tail[0]], tuple(tail)))
        base += 128 * tw
    vals_len = base

    # --- output segments: same descending-depth order as the acc region,
    # so any run of deepest classes is contiguous in both acc and out ---
    out_off = {}
    o = 0
    for d, c in cl:
        out_off[d] = o
        o += 128 * c
    out_len = o

    # --- stores: ascending depth, one DMA per class while wide enough;
    # the deep remainder (including any narrow shallow classes) becomes one
    # DMA over acc cols [0, A[d]+c) gated on the last TT ---
    # --- store split: per-class stores for wide shallow classes, one merged
    # store for the deep remainder (acc cols [0, X)) ---
    seg = {}
    # seg[d] = (base, row_stride, col_off): element (p, j) of class d lands
    # at DRAM base + p*row_stride + col_off + j
    asc = sorted(cl, key=lambda x: x[0])
    split = len(asc)
    for i, (d, c) in enumerate(asc):
        rem = sum(cc for _, cc in asc[i:])
        if c < MERGE_STORE_COLS or rem < MERGE_STORE_COLS:
            split = i
            break
    X = 0  # deep-region columns, owned by the Pool accumulation chain
    if split < len(asc):
        # widen the merged region until its rows clear the 512B/descriptor
        # threshold (below it the DMA bus pays a 2x latency multiplier)
        while split > 0 and A[asc[split][0]] + asc[split][1] < MERGE_STORE_COLS:
            split -= 1
        d0, c0 = asc[split]
        X = A[d0] + c0  # this class plus everything deeper
        for dd, cc in asc[split:]:
            seg[dd] = (0, X, A[dd])
    for d, c in asc[:split]:
        seg[d] = (out_off[d], c, 0)

    # --- two independent accumulation chains over disjoint column ranges:
    # DVE folds cols [X, W[l]) per layer (only shallow-class layers reach
    # there), Pool folds cols [0, min(W[l], X)). Column-disjoint => no
    # cross-engine hazard; each chain is RAW-ordered by its own semaphore.
    dve_tts = [(l, Wl[l] - X) for l in range(1, maxd) if Wl[l] > X]
    pool_tts = [(l, min(Wl[l], X)) for l in range(1, maxd) if X > 0]

    # stores: ("dve"|"pool", gate_count, sb_col, ncols, out_base)
    stores = []
    for d, c in asc[:split]:
        gate = sum(1 for l, _ in dve_tts if l <= d - 1)
        stores.append(("dve", gate, A[d], c, out_off[d]))
    if X > 0:
        stores.append(("pool", len(pool_tts), 0, X, 0))

    return {
        "cl": cl, "A": A, "W0": W0, "W": Wl, "sb_off": sb_off, "totf": totf,
        "BO": BO, "RW": RW, "loads": loads, "stores": stores, "seg": seg,
        "vals_len": vals_len, "out_len": out_len, "out_off": out_off,
        "X": X, "dve_tts": dve_tts, "pool_tts": pool_tts,
    }


def plan(vid2fill, patches, queryInds):
    """Host-side plan: class layout + per-core packed fp16 values + metadata."""
    vid2fill = np.asarray(vid2fill, dtype=np.float32)
    patches = np.asarray(patches, dtype=np.float32)
    queryInds = np.asarray(queryInds, dtype=np.int64)

    base_nonzero = bool(np.any(vid2fill))
    vid_cl = np.ascontiguousarray(vid2fill.transpose(0, 2, 3, 1))  # [T,H,W,C]

    core_of = queryInds[:, 0] // FPC
    core_data = []
    for k in range(NCORES):
        sel = core_of == k
        q_k = queryInds[sel].copy()
        q_k[:, 0] -= k * FPC
        base_k = (
            vid_cl[k * FPC : (k + 1) * FPC].reshape(-1) if base_nonzero else None
        )
        core_data.append(_prep_core(patches[sel], q_k, base_k))

    # device classes (depth >= 2), padded to the max across cores
    max_d = max(cd[5].shape[0] - 1 for cd in core_data)
    class_list = []
    for d in range(MIN_DEV_CLASS, max_d + 1):
        n = max(int(cd[5][d]) if d < cd[5].shape[0] else 0 for cd in core_data)
        if n == 0:
            continue
        cols = (n + P - 1) // P
        class_list.append((d, cols))
    class_list.sort(key=lambda x: -x[0])  # descending depth (prefix property)

    lay = _layout(class_list)
    A, BO, RW = lay["A"], lay["BO"], lay["RW"]

    per_core_vals = []
    per_core_meta = []
    for es, vs, rank, elem_class, pos_in_class, class_sizes in core_data:
        vals = np.zeros(lay["vals_len"], dtype=np.float16)
        dcls = elem_class[es]
        posc = pos_in_class[es]
        for d, cols in class_list:
            m = dcls == d
            if not m.any():
                continue
            pc = posc[m]
            r = rank[m]
            # layer-major: value of (class d, layer r, pos pc) lives in the
            # block holding layer r at [p = pc//cols, col = A[d] + pc%cols]
            bo = np.zeros(r.shape[0], dtype=np.int64)
            rw = np.zeros(r.shape[0], dtype=np.int64)
            for l in range(d):
                lm = r == l
                bo[lm] = BO[l]
                rw[lm] = RW[l]
            vals[bo + (pc // cols) * rw + A[d] + pc % cols] = vs[m]
        # depth-1 singleton values, addressed by element index (exact f32)
        single = dcls == 1
        per_core_vals.append(vals)
        per_core_meta.append(
            (elem_class, pos_in_class, es[single], vs[single])
        )
    return {
        "class_list": class_list,
        "layout": lay,
        "per_core_vals": per_core_vals,
        "per_core_meta": per_core_meta,
        "base_nonzero": base_nonzero,
        "vid_cl": vid_cl,
    }


def build_nc(class_list, lay=None):
    """Raw-Bass SPMD program, layer-major fp16: acc region = classes
    descending by depth; one wide in-place tensor_add per layer over the
    prefix that has that layer; each class's slice stored as soon as its
    last layer folds (small deep classes merged into one trailing store)."""
    if lay is None:
        lay = _layout(class_list)
    W0, Wl, sb_off, totf = lay["W0"], lay["W"], lay["sb_off"], lay["totf"]
    maxd = lay["cl"][0][0]
    nc = bass.Bass()
    f16 = mybir.dt.float16
    vals_t = nc.dram_tensor("vals", [lay["vals_len"]], f16, kind="ExternalInput")
    out_t = nc.dram_tensor("out", [lay["out_len"]], f16, kind="ExternalOutput")

    X = lay["X"]
    dve_tts = lay["dve_tts"]
    pool_tts = lay["pool_tts"]
    # which load DMA (by index) supplies each layer
    load_of_layer = {}
    for i, (_, _, _, ls) in enumerate(lay["loads"]):
        for l in ls:
            load_of_layer[l] = i

    with ExitStack() as ctx:
        sb = ctx.enter_context(nc.sbuf_tensor([P, totf], f16))
        ld_sem = {
            i: ctx.enter_context(nc.semaphore(name=f"ld_sem_{i}"))
            for i in range(len(lay["loads"]))
        }
        st_sem = ctx.enter_context(nc.semaphore(name="st_sem"))
        dve_sem = ctx.enter_context(nc.semaphore(name="dve_sem"))
        pool_sem = (
            ctx.enter_context(nc.semaphore(name="pool_sem")) if pool_tts else None
        )
        block = ctx.enter_context(nc.Block())

        @block.sync
        def _(sync):
            for i, (base, row_w, sb_col, ls) in enumerate(lay["loads"]):
                src = vals_t[base : base + 128 * row_w].rearrange(
                    "(p x) -> p x", p=P
                )
                sync.dma_start(
                    sb[:, sb_col : sb_col + row_w], src
                ).then_inc(ld_sem[i], 16)
            for eng, gate_cnt, sb_col, ncols, out_base in lay["stores"]:
                sync.wait_ge(dve_sem if eng == "dve" else pool_sem, gate_cnt)
                dst = out_t[out_base : out_base + 128 * ncols].rearrange(
                    "(p x) -> p x", p=P
                )
                sync.dma_start(
                    dst, sb[:, sb_col : sb_col + ncols]
                ).then_inc(st_sem, 16)

        @block.vector
        def _(vector):
            for i, (l, w) in enumerate(dve_tts):
                if i > 0:
                    vector.wait_ge(dve_sem, i)  # in-place RAW chain
                vector.wait_ge(ld_sem[load_of_layer[l]], 16)
                nc.vector.tensor_add(
                    out=sb[:, X : X + w],
                    in0=sb[:, X : X + w],
                    in1=sb[:, sb_off[l] + X : sb_off[l] + X + w],
                ).then_inc(dve_sem, 1)

        if pool_tts:

            @block.gpsimd
            def _(gp):
                for i, (l, w) in enumerate(pool_tts):
                    if i > 0:
                        gp.wait_ge(pool_sem, i)  # in-place RAW chain
                    gp.wait_ge(ld_sem[load_of_layer[l]], 16)
                    nc.gpsimd.tensor_add(
                        out=sb[:, 0:w],
                        in0=sb[:, 0:w],
                        in1=sb[:, sb_off[l] : sb_off[l] + w],
                    ).then_inc(pool_sem, 1)

    return nc


_NC_CACHE = {}


def kernel(vid2fill, patches, queryInds):
    pl = plan(vid2fill, patches, queryInds)
    class_list = pl["class_list"]
    lay = pl["layout"]

    key = tuple(class_list)
    if key not in _NC_CACHE:
        _NC_CACHE[key] = build_nc(class_list, lay)
    nc = _NC_CACHE[key]

    in_maps = [{"vals": pl["per_core_vals"][k]} for k in range(NCORES)]
    res = run_bass_kernel_spmd(nc, in_maps, core_ids=list(range(NCORES)))

    seg = lay["seg"]

    vid_cl = pl["vid_cl"]
    full = np.empty((T, H, W, C), dtype=np.float32)
    for k in range(NCORES):
        elem_class, pos_in_class, single_e, single_v = pl["per_core_meta"][k]
        dev = res.results[k]["out"].astype(np.float32)
        core_out = np.empty(NELEM, dtype=np.float32)
        # depth 0: base only (with a nonzero base it was folded in, so
        # depth 0 then means a true zero — vid_cl there is what we want
        # only when the base was NOT folded; when folded, depth>=1 always)
        zero_m = elem_class == 0
        core_out[zero_m] = vid_cl[k * FPC : (k + 1) * FPC].reshape(-1)[zero_m]
        # depth 1: the single contribution, no addition needed (exact f32)
        core_out[single_e] = single_v
        # depth >= 2: device-reduced
        dev_m = elem_class >= MIN_DEV_CLASS
        idx = np.zeros(NELEM, dtype=np.int64)
        for d, cols in class_list:
            m = elem_class == d
            base, stride, coloff = seg[d]
            pc = pos_in_class[m]
            idx[m] = base + (pc // cols) * stride + coloff + pc % cols
        core_out[dev_m] = dev[idx[dev_m]]
        full[k * FPC : (k + 1) * FPC] = core_out.reshape(FPC, H, W, C)

    return np.ascontiguousarray(full.transpose(0, 3, 1, 2))


# revision 25
# speedup vs baseline: 1.0117x; 1.0117x over previous
"""Scatter-add (col2im at random query corners) on 8 Trainium2 NeuronCores.

Problem: out[t,c,h+dh,w+dw] += patches[n,0,c,dh,dw] for each query n at
corner (t,h,w), on top of the vid2fill base. PT=1, so every patch touches
exactly one frame: shard by frame pairs (core k owns frames 2k, 2k+1); the
cores are fully independent, no collective needed.

Strategy ("depth-class compaction"): the host computes each output
element's contributor count (its depth d), groups output elements by d,
and lays the patch values out per class d as layer-major [128, d, n_d/128]
blocks — a pure permutation/padding of the input values (quantized to
fp16; the 2e-2 tolerance leaves ~16x margin over fp16's ~1.2e-3 worst
case). The device, per layer, streams one contiguous DMA load and performs
one in-place full-partition vector add over the layer slice, then stores
each class's slice once its last layer has folded. Every addition of the
scatter-add happens on-device as a dense, full-bandwidth op — the
memory-regime optimum (device traffic ~= fp16 patch bytes + output bytes).

fp16 halves DMA traffic vs f32; trailing small layers are merged into one
load block and deep small classes into one store so every DMA keeps >=512
contiguous bytes per descriptor (below that the DMA bus pays a 2x latency
multiplier).

Elements with depth 0 (base only) and depth 1 (a single contribution, no
addition required anywhere) are routed by the host during unpermutation at
full f32 precision.
"""

import sys
from contextlib import ExitStack

for _p in ("/opt/trn_rl_repo", "/root/.axon_site/_ro/trn_rl_repo"):
    if _p not in sys.path:
        sys.path.append(_p)

import numpy as np

import concourse.bass as bass
from concourse import mybir
from concourse.bass_utils import run_bass_kernel_spmd

T, C, H, W = 16, 3, 512, 512
PS, PT = 7, 1
NCORES = 8
FPC = T // NCORES          # frames per core
NPIX = FPC * H * W         # pixels per core
NELEM = NPIX * C           # channels-last elements per core
P = 128                    # SBUF partitions
MIN_DEV_CLASS = 2          # depth-1 elements need no addition; host routes them
MERGE_W = 512              # merge layer loads narrower than this (cols)
MERGE_STORE_COLS = 256     # merge deepest classes until stores are this wide


def _prep_core(patches_k, q_k, base_k):
    """Per-core contribution stream + depth classes (host, pure indexing)."""
    h = q_k[:, 1]
    w = q_k[:, 2]
    lt = q_k[:, 0]

    dh = np.arange(PS, dtype=np.int64)
    dw = np.arange(PS, dtype=np.int64)
    ch = np.arange(C, dtype=np.int64)
    # channels-last element index, axis order (n, c, dh, dw) = patches order
    pix = (lt[:, None, None] * H + (h[:, None, None] + dh[None, :, None])) * W + (
        w[:, None, None] + dw[None, None, :]
    )
    e = (pix[:, None, :, :] * C + ch[None, :, None, None]).reshape(-1)
    v = patches_k.reshape(-1)

    if base_k is not None:
        # fold the base video in as one extra contribution per element
        e = np.concatenate([e, np.arange(NELEM, dtype=np.int64)])
        v = np.concatenate([v, base_k.reshape(-1)])

    cnt = np.bincount(e, minlength=NELEM)          # depth per element
    order = np.argsort(e, kind="stable")
    es = e[order]
    vs = v[order]
    grp_start = np.cumsum(cnt) - cnt
    rank = np.arange(es.shape[0], dtype=np.int64) - grp_start[es]

    elem_class = cnt
    max_d = int(cnt.max()) if cnt.size else 0
    class_sizes = np.bincount(elem_class, minlength=max_d + 1)
    pos_in_class = np.empty(NELEM, dtype=np.int64)
    cls_order = np.argsort(elem_class, kind="stable")
    cls_starts = np.cumsum(class_sizes) - class_sizes
    pos_in_class[cls_order] = np.arange(NELEM, dtype=np.int64) - cls_starts[
        elem_class[cls_order]
    ]
    return es, vs, rank, elem_class, pos_in_class, class_sizes


def _layout(class_list):
    """Layer-major fp16 layout. class_list must be sorted descending by depth.

    Returns a dict with:
      cl          class list (descending depth)
      A[d]        acc-region column of class d
      W0          total acc columns
      W[l]        columns of layer l (classes with depth >= l+1), l=1..maxd-1
      sb_off[l]   SBUF column of layer l's staging slice (l>=1); acc at 0
      totf        total SBUF columns (acc + staging)
      BO[l]       DRAM element offset of layer l's partition-0 row (l=0..)
      RW[l]       DRAM row stride (elements) of the block holding layer l
      loads       [(dram_base, row_w, sb_col, layers)] one DMA per entry
      stores      [(gate_layer, sb_col, ncols, out_base)] one DMA per entry
      vals_len    DRAM input elements (fp16)
      out_len     DRAM output elements (fp16)
      out_off[d]  DRAM output element offset of class d's segment
    """
    cl = list(class_list)
    A = {}
    off = 0
    for d, c in cl:
        A[d] = off
        off += c
    W0 = off
    maxd = cl[0][0]
    Wl = {l: sum(c for d, c in cl if d >= l + 1) for l in range(1, maxd)}

    sb_off = {}
    off = W0
    for l in range(1, maxd):
        sb_off[l] = off
        off += Wl[l]
    totf = off

    # --- load blocks ---
    # block0 holds layers 0+1 (every device element's first two values),
    # row-interleaved so one DMA fills acc + the first staging slice.
    BO = {0: 0, 1: W0}
    RW = {0: W0 + Wl[1], 1: W0 + Wl[1]}
    loads = [(0, W0 + Wl[1], 0, (0, 1))]
    base = 128 * (W0 + Wl[1])
    rest = [l for l in range(2, maxd)]
    head = [l for l in rest if Wl[l] >= MERGE_W]
    tail = [l for l in rest if Wl[l] < MERGE_W]
    for l in head:
        BO[l] = base
        RW[l] = Wl[l]
        loads.append((base, Wl[l], sb_off[l], (l,)))
        base += 128 * Wl[l]
    if tail:
        tw = sum(Wl[l] for l in tail)
        pref = 0
        for l in tail:
            BO[l] = base + pref
            RW[l] = tw
            pref += Wl[l]
        loads.append((base, tw, sb_off[tail[0]], tuple(tail)))
        base += 128 * tw
    vals_len = base

    # --- output segments: same descending-depth order as the acc region,
    # so any run of deepest classes is contiguous in both acc and out ---
    out_off = {}
    o = 0
    for d, c in cl:
        out_off[d] = o
        o += 128 * c
    out_len = o

    # --- stores: ascending depth, one DMA per class while wide enough;
    # the deep remainder (including any narrow shallow classes) becomes one
    # DMA over acc cols [0, A[d]+c) gated on the last TT ---
    # --- store split: per-class stores for wide shallow classes, one merged
    # store for the deep remainder (acc cols [0, X)) ---
    seg = {}
    # seg[d] = (base, row_stride, col_off): element (p, j) of class d lands
    # at DRAM base + p*row_stride + col_off + j
    asc = sorted(cl, key=lambda x: x[0])
    split = len(asc)
    for i, (d, c) in enumerate(asc):
        rem = sum(cc for _, cc in asc[i:])
        if c < MERGE_STORE_COLS or rem < MERGE_STORE_COLS:
            split = i
            break
    # X: columns owned by the Pool accumulation chain (deepest classes only,
    # to keep the slow gpsimd chain short). S: columns of the final merged
    # store, widened past 256 cols so its rows clear the 512B/descriptor
    # threshold (below it the DMA bus pays a 2x latency multiplier). Any
    # class boundary X <= S works: classes in [X, S) still get all their
    # layers from the DVE chain.
    X = 0
    if split < len(asc):
        d0, c0 = asc[split]
        X = A[d0] + c0  # this class plus everything deeper
        split_s = split
        while split_s > 0 and A[asc[split_s][0]] + asc[split_s][1] < MERGE_STORE_COLS:
            split_s -= 1
        dS, cS = asc[split_s]
        S = A[dS] + cS
        for dd, cc in asc[split_s:]:
            seg[dd] = (0, S, A[dd])
    else:
        split_s = split
    for d, c in asc[:split_s]:
        seg[d] = (out_off[d], c, 0)

    # --- two independent accumulation chains over disjoint column ranges:
    # DVE folds cols [X, W[l]) per layer (only shallow-class layers reach
    # there), Pool folds cols [0, min(W[l], X)). Column-disjoint => no
    # cross-engine hazard; each chain is RAW-ordered by its own semaphore.
    dve_tts = [(l, Wl[l] - X) for l in range(1, maxd) if Wl[l] > X]
    pool_tts = [(l, min(Wl[l], X)) for l in range(1, maxd) if X > 0]

    # stores: (gates, sb_col, ncols, out_base) with gates a list of
    # ("dve"|"pool", count) pairs that must all be satisfied first
    stores = []
    for d, c in asc[:split_s]:
        gate = sum(1 for l, _ in dve_tts if l <= d - 1)
        stores.append(([("dve", gate)], A[d], c, out_off[d]))
    if X > 0:
        gates = [("pool", len(pool_tts))]
        if split_s < split:
            # classes in [X, S) take their layers from the DVE chain; the
            # deepest of them dictates how many DVE TTs must have run
            need = max(dd for dd, _ in asc[split_s:split]) - 1
            gates.append(("dve", sum(1 for l, _ in dve_tts if l <= need)))
        stores.append((gates, 0, S, 0))

    return {
        "cl": cl, "A": A, "W0": W0, "W": Wl, "sb_off": sb_off, "totf": totf,
        "BO": BO, "RW": RW, "loads": loads, "stores": stores, "seg": seg,
        "vals_len": vals_len, "out_len": out_len, "out_off": out_off,
        "X": X, "dve_tts": dve_tts, "pool_tts": pool_tts,
    }


def plan(vid2fill, patches, queryInds):
    """Host-side plan: class layout + per-core packed fp16 values + metadata."""
    vid2fill = np.asarray(vid2fill, dtype=np.float32)
    patches = np.asarray(patches, dtype=np.float32)
    queryInds = np.asarray(queryInds, dtype=np.int64)

    base_nonzero = bool(np.any(vid2fill))
    vid_cl = np.ascontiguousarray(vid2fill.transpose(0, 2, 3, 1))  # [T,H,W,C]

    core_of = queryInds[:, 0] // FPC
    core_data = []
    for k in range(NCORES):
        sel = core_of == k
        q_k = queryInds[sel].copy()
        q_k[:, 0] -= k * FPC
        base_k = (
            vid_cl[k * FPC : (k + 1) * FPC].reshape(-1) if base_nonzero else None
        )
        core_data.append(_prep_core(patches[sel], q_k, base_k))

    # device classes (depth >= 2), padded to the max across cores
    max_d = max(cd[5].shape[0] - 1 for cd in core_data)
    class_list = []
    for d in range(MIN_DEV_CLASS, max_d + 1):
        n = max(int(cd[5][d]) if d < cd[5].shape[0] else 0 for cd in core_data)
        if n == 0:
            continue
        cols = (n + P - 1) // P
        class_list.append((d, cols))
    class_list.sort(key=lambda x: -x[0])  # descending depth (prefix property)

    lay = _layout(class_list)
    A, BO, RW = lay["A"], lay["BO"], lay["RW"]

    per_core_vals = []
    per_core_meta = []
    for es, vs, rank, elem_class, pos_in_class, class_sizes in core_data:
        vals = np.zeros(lay["vals_len"], dtype=np.float16)
        dcls = elem_class[es]
        posc = pos_in_class[es]
        for d, cols in class_list:
            m = dcls == d
            if not m.any():
                continue
            pc = posc[m]
            r = rank[m]
            # layer-major: value of (class d, layer r, pos pc) lives in the
            # block holding layer r at [p = pc//cols, col = A[d] + pc%cols]
            bo = np.zeros(r.shape[0], dtype=np.int64)
            rw = np.zeros(r.shape[0], dtype=np.int64)
            for l in range(d):
                lm = r == l
                bo[lm] = BO[l]
                rw[lm] = RW[l]
            vals[bo + (pc // cols) * rw + A[d] + pc % cols] = vs[m]
        # depth-1 singleton values, addressed by element index (exact f32)
        single = dcls == 1
        per_core_vals.append(vals)
        per_core_meta.append(
            (elem_class, pos_in_class, es[single], vs[single])
        )
    return {
        "class_list": class_list,
        "layout": lay,
        "per_core_vals": per_core_vals,
        "per_core_meta": per_core_meta,
        "base_nonzero": base_nonzero,
        "vid_cl": vid_cl,
    }


def build_nc(class_list, lay=None):
    """Raw-Bass SPMD program, layer-major fp16: acc region = classes
    descending by depth; one wide in-place tensor_add per layer over the
    prefix that has that layer; each class's slice stored as soon as its
    last layer folds (small deep classes merged into one trailing store)."""
    if lay is None:
        lay = _layout(class_list)
    W0, Wl, sb_off, totf = lay["W0"], lay["W"], lay["sb_off"], lay["totf"]
    maxd = lay["cl"][0][0]
    nc = bass.Bass()
    f16 = mybir.dt.float16
    vals_t = nc.dram_tensor("vals", [lay["vals_len"]], f16, kind="ExternalInput")
    out_t = nc.dram_tensor("out", [lay["out_len"]], f16, kind="ExternalOutput")

    X = lay["X"]
    dve_tts = lay["dve_tts"]
    pool_tts = lay["pool_tts"]
    # which load DMA (by index) supplies each layer
    load_of_layer = {}
    for i, (_, _, _, ls) in enumerate(lay["loads"]):
        for l in ls:
            load_of_layer[l] = i

    with ExitStack() as ctx:
        sb = ctx.enter_context(nc.sbuf_tensor([P, totf], f16))
        ld_sem = {
            i: ctx.enter_context(nc.semaphore(name=f"ld_sem_{i}"))
            for i in range(len(lay["loads"]))
        }
        st_sem = ctx.enter_context(nc.semaphore(name="st_sem"))
        dve_sem = ctx.enter_context(nc.semaphore(name="dve_sem"))
        pool_sem = (
            ctx.enter_context(nc.semaphore(name="pool_sem")) if pool_tts else None
        )
        block = ctx.enter_context(nc.Block())

        @block.sync
        def _(sync):
            for i, (base, row_w, sb_col, ls) in enumerate(lay["loads"]):
                src = vals_t[base : base + 128 * row_w].rearrange(
                    "(p x) -> p x", p=P
                )
                sync.dma_start(
                    sb[:, sb_col : sb_col + row_w], src
                ).then_inc(ld_sem[i], 16)
            for gates, sb_col, ncols, out_base in lay["stores"]:
                for eng, gate_cnt in gates:
                    sync.wait_ge(dve_sem if eng == "dve" else pool_sem, gate_cnt)
                dst = out_t[out_base : out_base + 128 * ncols].rearrange(
                    "(p x) -> p x", p=P
                )
                sync.dma_start(
                    dst, sb[:, sb_col : sb_col + ncols]
                ).then_inc(st_sem, 16)

        @block.vector
        def _(vector):
            for i, (l, w) in enumerate(dve_tts):
                if i > 0:
                    vector.wait_ge(dve_sem, i)  # in-place RAW chain
                vector.wait_ge(ld_sem[load_of_layer[l]], 16)
                nc.vector.tensor_add(
                    out=sb[:, X : X + w],
                    in0=sb[:, X : X + w],
                    in1=sb[:, sb_off[l] + X : sb_off[l] + X + w],
                ).then_inc(dve_sem, 1)

        if pool_tts:

            @block.gpsimd
            def _(gp):
                for i, (l, w) in enumerate(pool_tts):
                    if i > 0:
                        gp.wait_ge(pool_sem, i)  # in-place RAW chain
                    gp.wait_ge(ld_sem[load_of_layer[l]], 16)
                    nc.gpsimd.tensor_add(
                        out=sb[:, 0:w],
                        in0=sb[:, 0:w],
                        in1=sb[:, sb_off[l] : sb_off[l] + w],
                    ).then_inc(pool_sem, 1)

    return nc


_NC_CACHE = {}


def kernel(vid2fill, patches, queryInds):
    pl = plan(vid2fill, patches, queryInds)
    class_list = pl["class_list"]
    lay = pl["layout"]

    key = tuple(class_list)
    if key not in _NC_CACHE:
        _NC_CACHE[key] = build_nc(class_list, lay)
    nc = _NC_CACHE[key]

    in_maps = [{"vals": pl["per_core_vals"][k]} for k in range(NCORES)]
    res = run_bass_kernel_spmd(nc, in_maps, core_ids=list(range(NCORES)))

    seg = lay["seg"]

    vid_cl = pl["vid_cl"]
    full = np.empty((T, H, W, C), dtype=np.float32)
    for k in range(NCORES):
        elem_class, pos_in_class, single_e, single_v = pl["per_core_meta"][k]
        dev = res.results[k]["out"].astype(np.float32)
        core_out = np.empty(NELEM, dtype=np.float32)
        # depth 0: base only (with a nonzero base it was folded in, so
        # depth 0 then means a true zero — vid_cl there is what we want
        # only when the base was NOT folded; when folded, depth>=1 always)
        zero_m = elem_class == 0
        core_out[zero_m] = vid_cl[k * FPC : (k + 1) * FPC].reshape(-1)[zero_m]
        # depth 1: the single contribution, no addition needed (exact f32)
        core_out[single_e] = single_v
        # depth >= 2: device-reduced
        dev_m = elem_class >= MIN_DEV_CLASS
        idx = np.zeros(NELEM, dtype=np.int64)
        for d, cols in class_list:
            m = elem_class == d
            base, stride, coloff = seg[d]
            pc = pos_in_class[m]
            idx[m] = base + (pc // cols) * stride + coloff + pc % cols
        core_out[dev_m] = dev[idx[dev_m]]
        full[k * FPC : (k + 1) * FPC] = core_out.reshape(FPC, H, W, C)

    return np.ascontiguousarray(full.transpose(0, 3, 1, 2))


# revision 26
# speedup vs baseline: 1.0197x; 1.0079x over previous
"""Scatter-add (col2im at random query corners) on 8 Trainium2 NeuronCores.

Problem: out[t,c,h+dh,w+dw] += patches[n,0,c,dh,dw] for each query n at
corner (t,h,w), on top of the vid2fill base. PT=1, so every patch touches
exactly one frame: shard by frame pairs (core k owns frames 2k, 2k+1); the
cores are fully independent, no collective needed.

Strategy ("depth compaction", column-major): the host computes each output
element's contributor count (its depth d) and sorts device-handled
elements (d >= 2) by depth descending; element i lands at SBUF slot
(partition i%128, column i//128). Layer r of the accumulation then only
concerns the first W[r+1] columns (a prefix), so the device streams one
contiguous fp16 DMA load per layer block and performs one in-place
full-partition vector add per layer; column regions are stored as soon as
the layers covering them have folded. Every addition of the scatter-add
happens on-device as a dense, full-bandwidth op — the memory-regime
optimum (device traffic ~= fp16 patch bytes + output bytes; padding is
under one column per layer plus cross-core size spread).

The deepest columns [0, X) are accumulated by a parallel gpsimd (Pool)
chain over all layers while the DVE chain handles [X, W[l]); the ranges
are column-disjoint so the engines never race, and the slow serial tail
of tiny deep layers runs concurrently with the wide DVE adds instead of
gating the final store. fp16 halves DMA traffic vs f32 (the 2e-2
tolerance leaves ~16x margin over fp16's ~1.2e-3 worst case); trailing
small layer loads are merged into one block and the deepest store spans
>= 256 columns so every DMA keeps >= 512 contiguous bytes per descriptor
(below that the DMA bus pays a 2x latency multiplier).

Elements with depth 0 (base only) and depth 1 (a single contribution, no
addition required anywhere) are routed by the host during unpermutation at
full f32 precision.
"""

import sys
from contextlib import ExitStack

for _p in ("/opt/trn_rl_repo", "/root/.axon_site/_ro/trn_rl_repo"):
    if _p not in sys.path:
        sys.path.append(_p)

import numpy as np

import concourse.bass as bass
from concourse import mybir
from concourse.bass_utils import run_bass_kernel_spmd

T, C, H, W = 16, 3, 512, 512
PS, PT = 7, 1
NCORES = 8
FPC = T // NCORES          # frames per core
NPIX = FPC * H * W         # pixels per core
NELEM = NPIX * C           # channels-last elements per core
P = 128                    # SBUF partitions
MIN_DEV_CLASS = 2          # depth-1 elements need no addition; host routes them
MERGE_W = 512              # merge layer loads narrower than this (cols)
MERGE_STORE_COLS = 256     # merge deepest regions until stores are this wide


def _prep_core(patches_k, q_k, base_k):
    """Per-core contribution stream + depth classes (host, pure indexing)."""
    h = q_k[:, 1]
    w = q_k[:, 2]
    lt = q_k[:, 0]

    dh = np.arange(PS, dtype=np.int64)
    dw = np.arange(PS, dtype=np.int64)
    ch = np.arange(C, dtype=np.int64)
    # channels-last element index, axis order (n, c, dh, dw) = patches order
    pix = (lt[:, None, None] * H + (h[:, None, None] + dh[None, :, None])) * W + (
        w[:, None, None] + dw[None, None, :]
    )
    e = (pix[:, None, :, :] * C + ch[None, :, None, None]).reshape(-1)
    v = patches_k.reshape(-1)

    if base_k is not None:
        # fold the base video in as one extra contribution per element
        e = np.concatenate([e, np.arange(NELEM, dtype=np.int64)])
        v = np.concatenate([v, base_k.reshape(-1)])

    cnt = np.bincount(e, minlength=NELEM)          # depth per element
    order = np.argsort(e, kind="stable")
    es = e[order]
    vs = v[order]
    grp_start = np.cumsum(cnt) - cnt
    rank = np.arange(es.shape[0], dtype=np.int64) - grp_start[es]

    elem_class = cnt
    max_d = int(cnt.max()) if cnt.size else 0
    class_sizes = np.bincount(elem_class, minlength=max_d + 1)
    pos_in_class = np.empty(NELEM, dtype=np.int64)
    cls_order = np.argsort(elem_class, kind="stable")
    cls_starts = np.cumsum(class_sizes) - class_sizes
    pos_in_class[cls_order] = np.arange(NELEM, dtype=np.int64) - cls_starts[
        elem_class[cls_order]
    ]
    return es, vs, rank, elem_class, pos_in_class, class_sizes


def _layout(Wl, maxd):
    """fp16 layer-major layout from per-layer column widths.

    Wl[l] (l = 1..maxd-1) is the column width of accumulation layer l =
    max over cores of ceil(#elements with depth >= l+1 / 128); Wl[0] is an
    alias of Wl[1] (the acc region covers every device element). Widths are
    non-increasing in l.

    Returns a dict with:
      W0          acc-region columns (== Wl[1])
      W[l]        layer widths
      sb_off[l]   SBUF column of layer l's staging slice (l>=1); acc at 0
      totf        total SBUF columns (acc + staging)
      BO[l]       DRAM element offset of layer l's partition-0 row (l=0..)
      RW[l]       DRAM row stride (elements) of the block holding layer l
      loads       [(dram_base, row_w, sb_col, layers)] one DMA per entry
      X           Pool-chain columns (deepest region), 0 if none
      dve_tts     [(l, width)] DVE chain: cols [X, X+width) per layer
      pool_tts    [(l, width)] Pool chain: cols [0, width) per layer
      stores      [(gates, sb_col, ncols, out_base)], gates = [(eng, cnt)]
      regions     [(a, b)] store column ranges ascending (out_base = 128a)
      vals_len    DRAM input elements (fp16)
      out_len     DRAM output elements (fp16)
    """
    W0 = Wl[1]
    sb_off = {}
    off = W0
    for l in range(1, maxd):
        sb_off[l] = off
        off += Wl[l]
    totf = off

    # --- load blocks ---
    # block0 holds layers 0+1 (every device element's first two values),
    # row-interleaved so one DMA fills acc + the first staging slice.
    BO = {0: 0, 1: W0}
    RW = {0: W0 + Wl[1], 1: W0 + Wl[1]}
    loads = [(0, W0 + Wl[1], 0, (0, 1))]
    base = 128 * (W0 + Wl[1])
    rest = [l for l in range(2, maxd)]
    head = [l for l in rest if Wl[l] >= MERGE_W]
    tail = [l for l in rest if Wl[l] < MERGE_W]
    for l in head:
        BO[l] = base
        RW[l] = Wl[l]
        loads.append((base, Wl[l], sb_off[l], (l,)))
        base += 128 * Wl[l]
    if tail:
        tw = sum(Wl[l] for l in tail)
        pref = 0
        for l in tail:
            BO[l] = base + pref
            RW[l] = tw
            pref += Wl[l]
        loads.append((base, tw, sb_off[tail[0]], tuple(tail)))
        base += 128 * tw
    vals_len = base

    # --- engine split: Pool owns the deepest columns [0, X) across all
    # layers (short, slow chain hidden under the wide DVE adds); DVE owns
    # [X, W[l]) for the layers that reach past X. Column-disjoint => no
    # cross-engine hazard. Elements at cols >= X have depth <= L* (where
    # X = Wl[L*]), so their layers all live in the DVE chain.
    X = 0
    narrow = [l for l in range(2, maxd) if Wl[l] < MERGE_STORE_COLS]
    if narrow:
        X = Wl[narrow[0]]
    dve_tts = [(l, Wl[l] - X) for l in range(1, maxd) if Wl[l] > X]
    pool_tts = [(l, min(Wl[l], X)) for l in range(1, maxd) if X > 0]

    # --- stores: column region [Wl[d], Wl[d-1]) holds (at most) the
    # depth-d elements; it is final once the layers wider than its start
    # have folded. Walk ascending d while regions stay wide; the deep
    # remainder [0, S) becomes one store gated on both full chains. ---
    def ndve(a):
        return sum(1 for l, _ in dve_tts if Wl[l] > a)

    stores = []
    regions = []
    S = W0
    for d in range(2, maxd + 1):
        hi = Wl[d - 1]
        lo = Wl[d] if d < maxd else 0
        if hi - lo < MERGE_STORE_COLS or lo < MERGE_STORE_COLS:
            S = hi
            break
        stores.append(([("dve", ndve(lo))], lo, hi - lo, 128 * lo))
        regions.append((lo, hi))
        S = lo
    if S > 0:
        gates = []
        if pool_tts:
            gates.append(("pool", len(pool_tts)))
        if dve_tts:
            gates.append(("dve", len(dve_tts)))
        stores.append((gates, 0, S, 0))
        regions.append((0, S))
    regions.sort()
    out_len = 128 * W0

    return {
        "W0": W0, "W": Wl, "sb_off": sb_off, "totf": totf,
        "BO": BO, "RW": RW, "loads": loads, "stores": stores,
        "regions": regions, "X": X, "dve_tts": dve_tts, "pool_tts": pool_tts,
        "vals_len": vals_len, "out_len": out_len, "maxd": maxd,
    }


def plan(vid2fill, patches, queryInds):
    """Host-side plan: layer widths + per-core packed fp16 values + metadata."""
    vid2fill = np.asarray(vid2fill, dtype=np.float32)
    patches = np.asarray(patches, dtype=np.float32)
    queryInds = np.asarray(queryInds, dtype=np.int64)

    base_nonzero = bool(np.any(vid2fill))
    vid_cl = np.ascontiguousarray(vid2fill.transpose(0, 2, 3, 1))  # [T,H,W,C]

    core_of = queryInds[:, 0] // FPC
    core_data = []
    for k in range(NCORES):
        sel = core_of == k
        q_k = queryInds[sel].copy()
        q_k[:, 0] -= k * FPC
        base_k = (
            vid_cl[k * FPC : (k + 1) * FPC].reshape(-1) if base_nonzero else None
        )
        core_data.append(_prep_core(patches[sel], q_k, base_k))

    maxd = max(cd[5].shape[0] - 1 for cd in core_data)
    # Nge[k][d] = #elements of core k with depth >= d; class_start[k][d] =
    # Nge[k][d+1] = sorted position where depth-d elements begin
    nge = np.zeros((NCORES, maxd + 2), dtype=np.int64)
    for k, cd in enumerate(core_data):
        cs = cd[5]
        for d in range(maxd, MIN_DEV_CLASS - 1, -1):
            nd = int(cs[d]) if d < cs.shape[0] else 0
            nge[k, d] = nd + nge[k, d + 1]
    Wl = {l: int(-(-nge[:, l + 1].max() // P)) for l in range(1, maxd)}
    Wl[0] = Wl[1]

    lay = _layout(Wl, maxd)
    BO, RW = lay["BO"], lay["RW"]

    per_core_vals = []
    per_core_meta = []
    for k, (es, vs, rank, elem_class, pos_in_class, class_sizes) in enumerate(
        core_data
    ):
        vals = np.zeros(lay["vals_len"], dtype=np.float16)
        dcls = elem_class[es]
        dev = dcls >= MIN_DEV_CLASS
        # sorted (depth-descending) position of each contribution's element
        srt = nge[k, dcls[dev] + 1] + pos_in_class[es[dev]]
        r = rank[dev]
        bo = np.zeros(r.shape[0], dtype=np.int64)
        rw = np.zeros(r.shape[0], dtype=np.int64)
        for l in range(maxd):
            lm = r == l
            if lm.any():
                bo[lm] = BO[l]
                rw[lm] = RW[l]
        vals[bo + (srt % P) * rw + srt // P] = vs[dev]
        single = dcls == 1
        per_core_vals.append(vals)
        per_core_meta.append(
            (elem_class, pos_in_class, nge[k], es[single], vs[single])
        )
    return {
        "widths": tuple(Wl[l] for l in range(maxd)),
        "maxd": maxd,
        "layout": lay,
        "per_core_vals": per_core_vals,
        "per_core_meta": per_core_meta,
        "base_nonzero": base_nonzero,
        "vid_cl": vid_cl,
    }


def build_nc(lay):
    """Raw-Bass SPMD program: per layer, one contiguous fp16 load and one
    in-place tensor_add over the layer's column prefix (DVE for cols
    [X, W[l]), Pool for [0, X)); column regions are stored as soon as the
    layers covering them have folded."""
    Wl, sb_off, totf = lay["W"], lay["sb_off"], lay["totf"]
    nc = bass.Bass()
    f16 = mybir.dt.float16
    vals_t = nc.dram_tensor("vals", [lay["vals_len"]], f16, kind="ExternalInput")
    out_t = nc.dram_tensor("out", [lay["out_len"]], f16, kind="ExternalOutput")

    X = lay["X"]
    dve_tts = lay["dve_tts"]
    pool_tts = lay["pool_tts"]
    # which load DMA (by index) supplies each layer
    load_of_layer = {}
    for i, (_, _, _, ls) in enumerate(lay["loads"]):
        for l in ls:
            load_of_layer[l] = i

    with ExitStack() as ctx:
        sb = ctx.enter_context(nc.sbuf_tensor([P, totf], f16))
        ld_sem = {
            i: ctx.enter_context(nc.semaphore(name=f"ld_sem_{i}"))
            for i in range(len(lay["loads"]))
        }
        st_sem = ctx.enter_context(nc.semaphore(name="st_sem"))
        dve_sem = ctx.enter_context(nc.semaphore(name="dve_sem"))
        pool_sem = (
            ctx.enter_context(nc.semaphore(name="pool_sem")) if pool_tts else None
        )
        block = ctx.enter_context(nc.Block())

        @block.sync
        def _(sync):
            for i, (base, row_w, sb_col, ls) in enumerate(lay["loads"]):
                src = vals_t[base : base + 128 * row_w].rearrange(
                    "(p x) -> p x", p=P
                )
                sync.dma_start(
                    sb[:, sb_col : sb_col + row_w], src
                ).then_inc(ld_sem[i], 16)
            for gates, sb_col, ncols, out_base in lay["stores"]:
                for eng, gate_cnt in gates:
                    sync.wait_ge(dve_sem if eng == "dve" else pool_sem, gate_cnt)
                dst = out_t[out_base : out_base + 128 * ncols].rearrange(
                    "(p x) -> p x", p=P
                )
                sync.dma_start(
                    dst, sb[:, sb_col : sb_col + ncols]
                ).then_inc(st_sem, 16)

        @block.vector
        def _(vector):
            for i, (l, w) in enumerate(dve_tts):
                if i > 0:
                    vector.wait_ge(dve_sem, i)  # in-place RAW chain
                vector.wait_ge(ld_sem[load_of_layer[l]], 16)
                nc.vector.tensor_add(
                    out=sb[:, X : X + w],
                    in0=sb[:, X : X + w],
                    in1=sb[:, sb_off[l] + X : sb_off[l] + X + w],
                ).then_inc(dve_sem, 1)

        if pool_tts:

            @block.gpsimd
            def _(gp):
                for i, (l, w) in enumerate(pool_tts):
                    if i > 0:
                        gp.wait_ge(pool_sem, i)  # in-place RAW chain
                    gp.wait_ge(ld_sem[load_of_layer[l]], 16)
                    nc.gpsimd.tensor_add(
                        out=sb[:, 0:w],
                        in0=sb[:, 0:w],
                        in1=sb[:, sb_off[l] : sb_off[l] + w],
                    ).then_inc(pool_sem, 1)

    return nc


_NC_CACHE = {}


def kernel(vid2fill, patches, queryInds):
    pl = plan(vid2fill, patches, queryInds)
    lay = pl["layout"]

    key = (pl["maxd"], pl["widths"])
    if key not in _NC_CACHE:
        _NC_CACHE[key] = build_nc(lay)
    nc = _NC_CACHE[key]

    in_maps = [{"vals": pl["per_core_vals"][k]} for k in range(NCORES)]
    res = run_bass_kernel_spmd(nc, in_maps, core_ids=list(range(NCORES)))

    # store region lookup tables (column -> region start/width/out_base)
    starts = np.array([a for a, b in lay["regions"]], dtype=np.int64)
    widths = np.array([b - a for a, b in lay["regions"]], dtype=np.int64)

    vid_cl = pl["vid_cl"]
    full = np.empty((T, H, W, C), dtype=np.float32)
    for k in range(NCORES):
        elem_class, pos_in_class, nge_k, single_e, single_v = pl["per_core_meta"][k]
        dev = res.results[k]["out"].astype(np.float32)
        core_out = np.empty(NELEM, dtype=np.float32)
        # depth 0: base only (with a nonzero base it was folded in, so
        # depth 0 then means a true zero — vid_cl there is what we want
        # only when the base was NOT folded; when folded, depth>=1 always)
        zero_m = elem_class == 0
        core_out[zero_m] = vid_cl[k * FPC : (k + 1) * FPC].reshape(-1)[zero_m]
        # depth 1: the single contribution, no addition needed (exact f32)
        core_out[single_e] = single_v
        # depth >= 2: device-reduced, at sorted position -> (p, col) ->
        # store-region flat offset
        dev_m = elem_class >= MIN_DEV_CLASS
        srt = nge_k[elem_class[dev_m] + 1] + pos_in_class[dev_m]
        p = srt % P
        col = srt // P
        ri = np.searchsorted(starts, col, side="right") - 1
        a = starts[ri]
        idx = 128 * a + p * widths[ri] + (col - a)
        core_out[dev_m] = dev[idx]
        full[k * FPC : (k + 1) * FPC] = core_out.reshape(FPC, H, W, C)

    return np.ascontiguousarray(full.transpose(0, 3, 1, 2))
